# revision 8
# baseline (speedup 1.0000x reference)
"""Trainium2 Bass kernel for nn_AvgPoolVectorsPerWSI (segment-mean over groups).

Math: x [N=2048, M=512, 7, 7], idx [N] in [0,64)
  out[g, m] = mean over {n: idx[n]==g} and spatial of x[n, m, :, :]  -> [64, 512, 1, 1]

Strategy (no collectives needed):
  - Shard over M: core k handles an m-slice of 64 channels. Each core reads
    its x slice [2048, 64, 49] (25.7 MB) once. The x stream runs at the
    16-SDMA-engine port limit (~26 GB/s/engine, 12.5 KB packets) -> ~62.5 us;
    everything else hides behind it except the last tile's drain.
  - All compute is fp32-exact. Per 128-row tile the work is split:
      * VectorE, m-channels [0, MV): spatial j-reduce to xs[n, m] (two ops of
        MV/2 channels so the reduce can start at half-tile granularity), then
        TensorE accumulates psum_small[g, m] += w[n, g]^T @ xs[n, m].
      * TensorE, m-channels [MV, 64): fused segment-sum directly on raw x,
        psum_big[g, (m,j)] += w[n, g]^T @ x[n, (m,j)] in four psum chunks.
    with w the scale-weighted one-hot (scale = 1/(count_g*49)), generated ON
    DEVICE by GpSimd from a 74 KB aux tensor so the HBM stream is just x.
  - Tail pipelining: tiles 14/15 are DMA'd in pieces (vec halves first, PE
    chunks last) so both engines chew the final bytes as they land. ScalarE
    copies psum_small -> out_sb (it has a PSUM port and is otherwise idle);
    VectorE j-reduces psum_big in three chunks as the last tile's matmul
    chunks complete. aux rides ScalarE's separate HWDGE queue so the x
    stream starts on the first sync-engine instruction.

Raw Block implementation (not Tile): the walrus matmul/DMA lowerings only
accept ONE attached sync-wait per instruction; standalone wait_ge
instructions sidestep that.

DMA-completion semaphores: tile t uses sem t % BUFS with a cumulative
threshold (16 per piece). A shared counter is only safe because a tile's sem
is reused (t+BUFS) strictly after tile t was consumed (the slot-reuse wait
orders the re-issue).
"""

from contextlib import ExitStack

import numpy as np

import concourse.bass as bass
import concourse.mybir as mybir
from concourse.bass_utils import run_bass_kernel_spmd

N = 2048          # samples
M = 512           # channels
HW = 49           # spatial (7*7)
G = 64            # groups
CORES = 8
ML = M // CORES   # 64 channels per core
F = ML * HW       # 3136 floats per (n, core)
P = 128           # partitions per tile
NT = N // P       # 16 n-tiles
BUFS = 8          # x-tile buffer depth == number of DMA semaphores

MV = 44           # m-channels through VectorE spatial-reduce + small matmul
MP = ML - MV      # 20 m-channels through TensorE raw fused matmul
FV = MV * HW      # 2156 vec-path columns
FP = MP * HW      # 980 raw columns through the PE
VH = (MV // 2) * HW  # 1078: half of the vec region (reduce granularity)

# fp32 matmul chunks must stay within one 2KB PSUM bank (bank 0 = cols
# [0:512), bank 1 = [512:980)); four chunks so the last tile's matmuls
# pipeline finely against the arriving pieces. PSUM accumulate bits are
# per-BANK on start=True (whole-bank clear), so only the FIRST chunk of each
# bank sets start at t=0 — it executes before the bank's other chunk's first
# write (program order), which then overwrites (bits clear) and accumulates
# from t=1 on.
CHUNKS = [(0, 256), (256, 512), (512, 768), (768, FP)]
NCH = len(CHUNKS)
CHUNK_START = {0, 2}  # chunk index that owns its bank's start=True at t=0

# epilogue j-reduce of psum_big in m-chunks. A PE write and a DVE read of the
# SAME psum bank is a fatal HW collision, so chunk (0,10) (bank 0 only, cols
# <= 490) waits for the last tile's bank-0 matmuls (chunk 1), and (10,20)
# (spans both banks) waits for ALL matmuls.
SUBRED = [
    (0, 10, (NT - 1) * NCH + 2),
    (10, MP, NT * NCH),
]

# DMA pieces per tile (column ranges within the tile's F columns). Tiles
# 0..13 stream as single 1.6MB transfers (12.5KB/partition packets = best
# SDMA efficiency); 14/15 are split so the tail pipelines.
PIECES = {t: [(0, F)] for t in range(NT)}
PIECES[NT - 2] = [(0, VH), (VH, FV), (FV, F)]
PIECES[NT - 1] = [(0, VH), (VH, FV)] + [(FV + lo, FV + hi) for lo, hi in CHUNKS]

# Multi-piece tiles need ONE SEMAPHORE PER PIECE: with several pieces in
# flight on one sem, the 16 SDMA engines' +1s interleave across pieces, so a
# cumulative count can be reached while an earlier piece is still in flight
# (engines drain their own stripes in order, but race each other).


F32 = mybir.dt.float32


def _build(wait_out=True):
    nc = bass.Bass(trn_type="TRN2", target_bir_lowering=False)
    x_ext = nc.declare_dram_parameter("x", [N, F], F32, isOutput=False)
    # aux[:, 0:64] iota row, aux[:, 64:128] scale row, aux[:, 128:144] idx
    aux_ext = nc.declare_dram_parameter("aux", [P, G + G + NT], F32,
                                        isOutput=False)
    out_ext = nc.declare_dram_parameter("out", [G, ML], F32, isOutput=True)

    x_t = x_ext.ap().rearrange("(t p) f -> t p f", p=P)  # [16, 128, 3136]

    with ExitStack() as ctx:
        x_buf = ctx.enter_context(nc.sbuf_tensor([P, BUFS * F], F32))
        xs_buf = ctx.enter_context(nc.sbuf_tensor([P, BUFS * MV], F32))
        aux_sb = ctx.enter_context(nc.sbuf_tensor([P, G + G + NT], F32))
        warm_sb = ctx.enter_context(nc.sbuf_tensor([G, 2], F32))
        w_sb = ctx.enter_context(nc.sbuf_tensor([P, NT * G], F32))
        out_sb = ctx.enter_context(nc.sbuf_tensor([G, ML], F32))
        psum_big = ctx.enter_context(nc.psum_tensor([G, FP], F32))
        psum_small = ctx.enter_context(nc.psum_tensor([G, MV], F32))
        dma_x = [
            ctx.enter_context(nc.semaphore(name=f"dma_x{s}"))
            for s in range(BUFS)
        ]
        # piece sems for the split tiles: dma_p[t][k] guards PIECES[t][k]
        dma_p = {
            t: [
                ctx.enter_context(nc.semaphore(name=f"dma_p{t}_{k}"))
                for k in range(len(PIECES[t]))
            ]
            for t in range(NT)
            if len(PIECES[t]) > 1
        }

        def xsem(t, k):
            # (sem, threshold) guarding piece k of tile t
            if len(PIECES[t]) == 1:
                return dma_x[t % BUFS], 16 * (t // BUFS + 1)
            return dma_p[t][k], 16

        def vec_wait(engine, t, half):
            sem, thr = xsem(t, half if len(PIECES[t]) > 1 else 0)
            engine.wait_ge(sem, thr)

        def pe_wait(engine, t, chunk):
            if len(PIECES[t]) == 1:
                k = 0
            elif t == NT - 2:
                k = 2
            else:
                k = 2 + chunk
            sem, thr = xsem(t, k)
            engine.wait_ge(sem, thr)
        dma_a = ctx.enter_context(nc.semaphore())   # +16 when aux resident
        dma_o = ctx.enter_context(nc.semaphore())   # +16 when out written
        wg_sem = ctx.enter_context(nc.semaphore())  # +1 when w generated
        red_sem = ctx.enter_context(nc.semaphore())  # +2 per tile j-reduce
        pe_big = ctx.enter_context(nc.semaphore())   # +1 per big matmul chunk
        pe_tile = ctx.enter_context(nc.semaphore())  # +1 per tile (small mm)
        fin_sem = ctx.enter_context(nc.semaphore())  # +3 when out_sb ready
        block = ctx.enter_context(nc.Block())

        # ---- DMA program for x + out (SP / HWDGE, FIFO) ----
        @block.sync
        def _(sync):
            for t in range(NT):
                if t >= BUFS:
                    # slot reuse: the small matmul is ordered after both the
                    # j-reduces and the big matmuls of its tile
                    sync.wait_ge(pe_tile, t - BUFS + 1)
                slot = t % BUFS
                for k, (lo, hi) in enumerate(PIECES[t]):
                    sem, _ = xsem(t, k)
                    sync.dma_start(
                        out=x_buf[:, slot * F + lo:slot * F + hi],
                        in_=x_t[t][:, lo:hi],
                    ).then_inc(sem, 16)
            sync.wait_ge(fin_sem, 3)
            sync.dma_start(out=out_ext.ap(), in_=out_sb[:, :]).then_inc(dma_o, 16)
            if wait_out:
                sync.wait_ge(dma_o, 16)

        # ---- ScalarE: aux DMA on the second HWDGE queue; psum_small copy ----
        @block.scalar
        def _(scalar):
            scalar.dma_start(out=aux_sb[:, :], in_=aux_ext.ap()).then_inc(dma_a, 16)
            # warm the ACT Copy PWP table now — the first activation triggers
            # a ~1.3us ACT_TABLE_LOAD which must not land on the final-copy
            # critical path
            scalar.copy(warm_sb[:, 0:1], warm_sb[:, 1:2])
            scalar.wait_ge(pe_tile, NT)
            scalar.copy(out_sb[:, 0:MV], psum_small[:, :]).then_inc(fin_sem, 1)

        # ---- VectorE: w generation, spatial j-reduction, psum_big epilogue ----
        @block.vector
        def _(vector):
            # generate the scale-weighted one-hot from idx:
            #   w[p, t*G+g] = (idx[t*128+p] == g) * scale[g]
            vector.wait_ge(dma_a, 16)
            for t in range(NT):
                wg = vector.scalar_tensor_tensor(
                    out=w_sb[:, t * G:(t + 1) * G],
                    in0=aux_sb[:, 0:G],
                    scalar=aux_sb[:, 2 * G + t:2 * G + t + 1],
                    in1=aux_sb[:, G:2 * G],
                    op0=mybir.AluOpType.is_equal,
                    op1=mybir.AluOpType.mult,
                )
            wg.then_inc(wg_sem, 1)

            for t in range(NT):
                slot = t % BUFS
                if t >= BUFS:
                    # xs slot reuse: wait until tile t-BUFS consumed by PE
                    vector.wait_ge(pe_tile, t - BUFS + 1)
                for half in range(2):
                    vec_wait(vector, t, half)
                    vector.tensor_reduce(
                        out=xs_buf[:, slot * MV + half * (MV // 2):
                                   slot * MV + (half + 1) * (MV // 2)],
                        in_=x_buf[:, slot * F + half * VH:
                                  slot * F + (half + 1) * VH].rearrange(
                            "p (m j) -> p m j", j=HW
                        ),
                        axis=mybir.AxisListType.X,
                        op=mybir.AluOpType.add,
                    ).then_inc(red_sem, 1)

            # epilogue: j-reduce psum_big in m-chunks as the last tile's
            # matmul chunks complete
            for mlo, mhi, need in SUBRED:
                vector.wait_ge(pe_big, need)
                vector.tensor_reduce(
                    out=out_sb[:, MV + mlo:MV + mhi],
                    in_=psum_big[:, mlo * HW:mhi * HW].rearrange(
                        "p (m j) -> p m j", j=HW
                    ),
                    axis=mybir.AxisListType.X,
                    op=mybir.AluOpType.add,
                ).then_inc(fin_sem, 1)

        # ---- TensorE: segment-sum accumulation (fp32) ----
        @block.tensor
        def _(tensor):
            tensor.wait_ge(wg_sem, 1)
            for t in range(NT):
                slot = t % BUFS
                wt = w_sb[:, t * G:(t + 1) * G]
                prev_key = None
                for c, (lo, hi) in enumerate(CHUNKS):
                    key = (t, 2 + c) if t == NT - 1 else (t, 0)
                    if key != prev_key:
                        pe_wait(tensor, t, c)
                        prev_key = key
                    tensor.matmul(
                        out=psum_big[:, lo:hi],
                        lhsT=wt,
                        rhs=x_buf[:, slot * F + FV + lo:slot * F + FV + hi],
                        start=(t == 0 and c in CHUNK_START),
                        stop=(t == NT - 1),
                        skip_group_check=True,
                    ).then_inc(pe_big, 1)
                tensor.wait_ge(red_sem, 2 * (t + 1))
                tensor.matmul(
                    out=psum_small[:, :],
                    lhsT=wt,
                    rhs=xs_buf[:, slot * MV:(slot + 1) * MV],
                    start=(t == 0),
                    stop=(t == NT - 1),
                ).then_inc(pe_tile, 1)

    return nc


def _prepare(x, idx):
    x = np.asarray(x)
    if x.dtype != np.float32:
        x = x.astype(np.float32)
    idx = np.asarray(idx).astype(np.int64)
    counts = np.bincount(idx, minlength=G).astype(np.float64)
    scale = np.where(counts > 0, 1.0 / (counts * HW), 0.0).astype(np.float32)
    aux = np.zeros((P, G + G + NT), np.float32)
    aux[:, 0:G] = np.arange(G, dtype=np.float32)[None, :]
    aux[:, G:2 * G] = scale[None, :]
    aux[:, 2 * G:] = idx.reshape(NT, P).T.astype(np.float32)
    xr = x.reshape(N, M, HW)
    in_maps = []
    for k in range(CORES):
        shard = np.ascontiguousarray(xr[:, k * ML:(k + 1) * ML, :]).reshape(N, F)
        in_maps.append({"x": shard, "aux": aux})
    return in_maps


def run(x, tensor_list_assignmentindices, trace=False, wait_out=True):
    in_maps = _prepare(x, tensor_list_assignmentindices)
    nc = _build(wait_out=wait_out)
    res = run_bass_kernel_spmd(nc, in_maps, core_ids=list(range(CORES)), trace=trace)
    outs = [np.asarray(r["out"]) for r in res.results]
    out = np.concatenate(outs, axis=1)  # [G, M]
    return out.reshape(G, M, 1, 1).astype(np.float32), res.exec_time_ns


def kernel(**inputs):
    out, _ = run(inputs["x"], inputs["tensor_list_assignmentindices"], trace=False)
    return out
